# revision 30
# baseline (speedup 1.0000x reference)
"""Cosine cross-attention (B=4, L=2048, D=1024, H=16, dh=64, tau=0.07) on 8 trn2 cores.

Sharding: core = b*2 + g  (b in 0..3 data-parallel, g in 0..1 head-group of 8 heads).

v3 layout — exp on ACT is the critical resource; the schedule starts the
exp stream as early as possible and hides every other engine behind it:

  Pre-exp (~30us, DMA/PE-paced): k-projection for head-pair m=0 only
  (pass 1 over the lb blocks), q-projection for lb=0 (all m), and half the
  V projection.  Norm rsqrt = exp(-0.5*ln(nsq)) on ACT — Ln and Exp share
  one activation table, so norm chains can interleave with the phase-C exp
  stream at zero table-reload cost.
  Phase C: 16 supertiles (m outer, lq inner).  Per (m, lq): 16 lk chunks of
  scores S.T = KT-chunk.T @ QT-block (bf16, two heads in the two PE halves),
  exp on ACT -> bf16 et ring (deep: PV may lag ~12us behind the exp
  stream), then PV into column-packed PSUM accumulators [V | 1].
  All remaining work — k pass 2 (m=1..3, re-streamed xk), q lb=1..3,
  leftover V projection, oa drains, PE transposes back to feature-major,
  and the out-projection — runs as filler tasks popped between lk chunks,
  budgeted to keep PE just under the ACT pace.
  PSUM: scores 2x[P,1024] + O-accum 2x[P,260] + 2-bank task ring = 8 banks.

Host: out[b] = (partial_g0 + partial_g1).T + (bo + bv @ Wo.T).
"""

import os

# some harnesses pin jax to cpu for the reference; this kernel needs the
# axon/neuron backend, so clear the pin before jax is first imported
if os.environ.get("JAX_PLATFORMS") == "cpu":
    del os.environ["JAX_PLATFORMS"]

import numpy as np

import concourse.bacc as bacc
import concourse.tile as tile
from concourse import mybir
from concourse.bass_utils import run_bass_kernel_spmd

P = 128
L = 2048
D = 1024
DO = 512  # per-core output dims of q/k/v projections (8 heads * 64)
TAU = 0.07
NLB = L // 512   # 4 blocks of 512 along L
NLK = L // 128   # 16 chunks of 128 along L (keys)
NM = DO // P     # 4 dout chunks (head pairs)
NKC = D // P     # 8 contraction chunks for projections

F32 = mybir.dt.float32
F32R = mybir.dt.float32r
BF16 = mybir.dt.bfloat16
E4 = mybir.dt.float8e4
E5 = mybir.dt.float8e5
DROW = mybir.MatmulPerfMode.DoubleRow
EXP = mybir.ActivationFunctionType.Exp
LN = mybir.ActivationFunctionType.Ln
MULT = mybir.AluOpType.mult

# how many V-projection lk-units run before the exp stream opens
N_V_PRE = 8

_CACHE = {}


def _emit(nc, prm, repeat=1):
    with tile.TileContext(nc) as tc:
        if repeat > 1:
            with tc.For_i(0, repeat, 1):
                _emit_body(nc, tc, prm)
        else:
            _emit_body(nc, tc, prm)


def _emit_body(nc, tc, prm):
    from collections import deque
    from contextlib import ExitStack
    with ExitStack() as stack:
        const = stack.enter_context(tc.tile_pool(name="const", bufs=1))
        persist = stack.enter_context(tc.tile_pool(name="persist", bufs=1))
        wqp = stack.enter_context(tc.tile_pool(name="wqp", bufs=1))
        wkp = stack.enter_context(tc.tile_pool(name="wkp", bufs=1))
        wvp = stack.enter_context(tc.tile_pool(name="wvp", bufs=1))
        wop = stack.enter_context(tc.tile_pool(name="wop", bufs=1))
        xkp = stack.enter_context(tc.tile_pool(name="xkp", bufs=4))
        xqp = stack.enter_context(tc.tile_pool(name="xqp", bufs=3))
        xvp = stack.enter_context(tc.tile_pool(name="xvp", bufs=2))
        sqp = stack.enter_context(tc.tile_pool(name="sqp", bufs=6))
        nrp = stack.enter_context(tc.tile_pool(name="nrp", bufs=2))
        etp = stack.enter_context(tc.tile_pool(name="etp", bufs=int(os.environ.get("K_ET", "10"))))
        mnp = stack.enter_context(tc.tile_pool(name="mnp", bufs=2))
        rzp = stack.enter_context(tc.tile_pool(name="rzp", bufs=4))
        obp = stack.enter_context(tc.tile_pool(name="obp", bufs=2))
        psS = stack.enter_context(tc.tile_pool(name="psS", bufs=2, space="PSUM"))
        psO = stack.enter_context(tc.tile_pool(name="psO", bufs=1, space="PSUM"))
        psDT = stack.enter_context(tc.tile_pool(name="psDT", bufs=2, space="PSUM"))

        # ---------------- persistent tiles ----------------
        qt = [persist.tile([P, L], BF16, tag=f"qt{m}", name=f"qt{m}")
              for m in range(NM)]
        kt = [persist.tile([P, L], BF16, tag=f"kt{m}", name=f"kt{m}")
              for m in range(NM)]
        vg_all = persist.tile([P, NLK, 8, 65], BF16, tag="vg_all")
        vg = [vg_all[:, i] for i in range(NLK)]
        nc.vector.memset(vg_all[:, :, :, 64], 1.0)  # softmax-Z ones column
        alias = not os.environ.get("K_NO_ALIAS")
        mt = [persist.tile([P, L], BF16,
                           tag=(f"kt{m - 1}" if (alias and m > 0) else f"mt{m}"),
                           name=f"mt{m}")
              for m in range(NM)]

        bq_t = const.tile([P, NM], F32, tag="bq")
        bk_t = const.tile([P, NM], F32, tag="bk")
        indt = const.tile([P, 2], BF16, tag="indt")
        sel2q = const.tile([66, P], F32R, tag="sel2q")
        sel2k = const.tile([66, P], F32R, tag="sel2k")
        ident = const.tile([P, P], BF16, tag="ident")

        # ---------------- DMA emission (per-queue ordering) ----------------
        # SP queue: wk, xk pass-1 stream, wq/xq, wv/xv.  Weights are
        # coalesced into 2 triggers each (trigger issue costs ~0.6us a piece
        # on the queue).
        wk2 = [wkp.tile([P, 4, DO], BF16, tag=f"wk{h}", name=f"wk{h}")
               for h in range(2)]
        wk_t = [wk2[kc // 4][:, kc % 4] for kc in range(NKC)]
        nc.sync.dma_start(out=wk2[0][:],
                          in_=prm["wk"][0:4 * P, :].rearrange(
                              "(c p) i -> p c i", p=P))

        def dma_xk(lb):
            t = xkp.tile([P, NKC, 512], BF16, tag="xk", name="xk")
            nc.sync.dma_start(
                out=t[:],
                in_=prm["xk"][:, lb * 512:(lb + 1) * 512].rearrange(
                    "(c p) i -> p c i", p=P))
            return t

        xk1 = [dma_xk(0)]
        nc.sync.dma_start(out=wk2[1][:],
                          in_=prm["wk"][4 * P:8 * P, :].rearrange(
                              "(c p) i -> p c i", p=P))
        def dma_xq(lb, eng):
            t = xqp.tile([P, NKC, 512], BF16, tag="xq", name="xq")
            eng.dma_start(
                out=t[:],
                in_=prm["xq"][:, lb * 512:(lb + 1) * 512].rearrange(
                    "(c p) i -> p c i", p=P))
            return t

        xq_t = [None] * NLB

        # ACT queue: consts, biases, wv, xv stream
        nc.scalar.dma_start(out=bq_t[:],
                            in_=prm["bq"][:].rearrange("(m p) -> p m", p=P))
        nc.scalar.dma_start(out=bk_t[:],
                            in_=prm["bk"][:].rearrange("(m p) -> p m", p=P))
        nc.scalar.dma_start(out=indt[:], in_=prm["indt"][:])
        nc.scalar.dma_start(out=sel2q[:], in_=prm["sel2q"][:])
        nc.scalar.dma_start(out=sel2k[:], in_=prm["sel2k"][:])
        nc.scalar.dma_start(out=ident[:], in_=prm["ident"][:])
        wq2 = [wqp.tile([P, 4, DO], BF16, tag=f"wq{h}", name=f"wq{h}")
               for h in range(2)]
        wq_t = [wq2[kc // 4][:, kc % 4] for kc in range(NKC)]
        nc.sync.dma_start(out=wq2[0][:],
                          in_=prm["wq"][0:4 * P, :].rearrange(
                              "(c p) i -> p c i", p=P))
        nc.sync.dma_start(out=wq2[1][:],
                          in_=prm["wq"][4 * P:8 * P, :].rearrange(
                              "(c p) i -> p c i", p=P))
        xq_t[0] = dma_xq(0, nc.sync)
        xk1.append(dma_xk(1))
        xk1.append(dma_xk(2))
        xk1.append(dma_xk(3))
        wv2 = [wvp.tile([P, 4, DO], BF16, tag=f"wv{h}", name=f"wv{h}")
               for h in range(2)]
        wvt = [wv2[kc // 4][:, kc % 4] for kc in range(NKC)]
        for h in range(2):
            nc.sync.dma_start(
                out=wv2[h][:],
                in_=prm["wv"][4 * h * P:(4 * h + 4) * P, :].rearrange(
                    "(c p) i -> p c i", p=P))
        def dma_xv(lb):
            t = xvp.tile([P, NKC, 512], BF16, tag="xv", name="xv")
            nc.sync.dma_start(
                out=t[:],
                in_=prm["xv"][:, lb * 512:(lb + 1) * 512].rearrange(
                    "(c p) i -> p c i", p=P))
            xv_t[lb] = t

        xv_t = [None] * NLB
        dma_xv(0)
        dma_xv(1)

        # xq1/xq2 take the two remaining xqp slots (first use, no lap);
        # they go at the END of the ACT trigger queue so the pre-exp
        # critical transfers run first.
        xq_t[1] = dma_xq(1, nc.sync)
        xq_t[2] = dma_xq(2, nc.sync)
        # xk pass-2 and xq3 lap ring slots: their DMAs are emitted inline in
        # the filler stream, after the previous occupant's readers.  wo is
        # not needed until the out-projection: also a filler.
        xk2 = [None] * NLB
        wot = [wop.tile([P, D], BF16, tag=f"wo{kc}", name=f"wo{kc}")
               for kc in range(NM)]

        def dma_wo():
            for kc in range(NM):
                nc.sync.dma_start(out=wot[kc][:],
                                  in_=prm["wo"][kc * P:(kc + 1) * P, :])

        # ---------------- unit helpers ----------------
        sq_store = {}

        def proj_unit(kind, m, lb):
            """projection + bias + square for one (kind, m, lb)."""
            w = wq_t if kind == "q" else wk_t
            x = (xq_t[lb] if kind == "q"
                 else (xk1[lb] if m == 0 else xk2[lb]))
            dst = qt if kind == "q" else kt
            b_t = bq_t if kind == "q" else bk_t
            sl = slice(lb * 512, (lb + 1) * 512)
            pa = psDT.tile([P, 512], F32, tag="dt")
            for kc in range(NKC):
                nc.tensor.matmul(pa[:], lhsT=w[kc][:, m * P:(m + 1) * P],
                                 rhs=x[:, kc, :],
                                 start=(kc == 0), stop=(kc == NKC - 1))
            blk = dst[m][:, sl]
            nc.vector.tensor_scalar_add(out=blk, in0=pa[:],
                                        scalar1=b_t[:, m:m + 1])
            sq = sqp.tile([P, 512], BF16, tag="sq")
            nc.gpsimd.tensor_tensor(out=sq[:], in0=blk, in1=blk, op=MULT)
            sq_store[(kind, m, lb)] = sq

        def norm_unit(kind, m_list, lb):
            """rsqrt of column norms for (kind, lb) over m_list, then scale
            the projected blocks in place.  rsqrt = exp(-0.5*ln(nsq)): Ln/Exp
            share an activation table with the phase-C exp stream."""
            sel = sel2q if kind == "q" else sel2k
            dst = qt if kind == "q" else kt
            sl = slice(lb * 512, (lb + 1) * 512)
            assert len(m_list) <= 3
            # PSUM matmul outs may only sit at partition bases 0/32/64: pack
            # one head-pair per base.  The Ln/Exp pass covers rows 0..65 in
            # one instruction each; the garbage rows in between are never
            # read (ACT cost depends only on the free size).
            nprt = 32 * (len(m_list) - 1) + 2
            psn = psDT.tile([P, 512], F32, tag="dt")
            if len(m_list) > 1:
                # the Ln below reads rows 0:nprt contiguously; prefill so no
                # stale/uninitialized PSUM is read (ln(1)=0 -> exp(0)=1).  The
                # matmuls then overwrite their 2-row bases.  Partition base
                # must be 0/32/64, so cover 0:64 wholesale.
                nc.vector.memset(psn[0:64, :], 1.0)
            for i, m in enumerate(m_list):
                sq = sq_store.pop((kind, m, lb))
                nc.tensor.matmul(psn[32 * i:32 * i + 2, :], lhsT=indt[:],
                                 rhs=sq[:], start=True, stop=True)
            nrl = nrp.tile([66, 512], F32, tag="nrl")
            nc.scalar.activation(out=nrl[0:nprt, :], in_=psn[0:nprt, :],
                                 func=LN)
            nr = nrp.tile([66, 512], F32R, tag="nr")
            nc.scalar.activation(out=nr[0:nprt, :], in_=nrl[0:nprt, :],
                                 func=EXP, scale=-0.5)
            for i, m in enumerate(m_list):
                bc = psDT.tile([P, 512], F32, tag="dt")
                nc.tensor.matmul(bc[:], lhsT=sel[32 * i:32 * i + 2, :],
                                 rhs=nr[32 * i:32 * i + 2, :],
                                 start=True, stop=True)
                blk = dst[m][:, sl]
                nc.vector.tensor_tensor(out=blk, in0=blk, in1=bc[:], op=MULT)

        def v_unit(lc):
            """V projection for one 128-token chunk lc (all 8 heads)."""
            lb, j = lc // 4, lc % 4
            pv = psDT.tile([P, 512], F32, tag="dt")
            for kc in range(NKC):
                nc.tensor.matmul(
                    pv[:], lhsT=xv_t[lb][:, kc, j * P:(j + 1) * P],
                    rhs=wvt[kc][:], start=(kc == 0), stop=(kc == NKC - 1))
            nc.vector.tensor_copy(
                out=vg[lc][:, :, 0:64],
                in_=pv[:].rearrange("p (h d) -> p h d", h=8))

        def make_drain(m, lq, oa):
            def drain():
                # 1/Z then per-partition scale into MN (natural [q, f]),
                # then PE transpose back to feature-major mt.
                mn = mnp.tile([P, 512], BF16, tag="mn")
                for s in range(2):
                    rz = rzp.tile([P, 4], F32, tag="rz")
                    with nc.allow_low_precision(reason="softmax Z reciprocal"):
                        nc.vector.reciprocal(out=rz[:], in_=oa[s][:, 64:260:65])
                    for c in range(4):
                        nc.vector.tensor_scalar_mul(
                            out=mn[:, (s * 4 + c) * 64:(s * 4 + c + 1) * 64],
                            in0=oa[s][:, c * 65:c * 65 + 64],
                            scalar1=rz[:, c:c + 1])
                psT = psDT.tile([P, 512], BF16, tag="dt")
                for s in range(2):
                    for c in range(4):
                        nc.tensor.transpose(
                            psT[s * 64:(s + 1) * 64, c * P:(c + 1) * P],
                            mn[:, (s * 4 + c) * 64:(s * 4 + c + 1) * 64],
                            ident[:])
                nc.vector.tensor_copy(
                    out=mt[m][:, lq * 512:(lq + 1) * 512], in_=psT[:])
            return drain

        def make_op(mo, lq):
            def op():
                pd = psDT.tile([P, 512], F32, tag="dt")
                for kc in range(NM):
                    nc.tensor.matmul(
                        pd[:], lhsT=wot[kc][:, mo * P:(mo + 1) * P],
                        rhs=mt[kc][:, lq * 512:(lq + 1) * 512],
                        start=(kc == 0), stop=(kc == NM - 1))
                ob = obp.tile([P, 512], BF16, tag="ob")
                nc.vector.tensor_copy(out=ob[:], in_=pd[:])
                nc.sync.dma_start(
                    out=prm["out_t"][mo * P:(mo + 1) * P,
                                     lq * 512:(lq + 1) * 512],
                    in_=ob[:])
            return op

        # ---------------- PE warm-up ----------------
        # The tensor engine needs ~3us of continuous work to reach full
        # clock; while the first DMAs stream in, run a burst of garbage
        # matmuls (operands memset, results never read) so the real
        # projections start at full speed.
        if not os.environ.get("K_NO_WARMUP"):
            warm = persist.tile([P, 512], BF16, tag="warm", name="warm")
            nc.gpsimd.memset(warm[:], 0.25)
            for i in range(14):
                pw = psDT.tile([P, 512], F32, tag="dt")
                nc.tensor.matmul(pw[:], lhsT=warm[:, 0:P], rhs=warm[:],
                                 start=True, stop=True)

        # ---------------- pre-exp phase ----------------
        # Just enough for supertile (m0, lq0) to open: k pass 1 for lb=0 and
        # the m0 part of q lb=0.  The rest of k pass 1 and q lb0 stream in as
        # the very first fillers — supertile 0's scores only need kt[0]'s lb
        # block at lk=4*lb, so the exp stream opens ~25us earlier.
        proj_unit("k", 0, 0)
        norm_unit("k", [0], 0)
        proj_unit("q", 0, 0)
        norm_unit("q", [0], 0)
        # V projection fully pre-exp: supertile PV matmuls are emitted inline,
        # so every vg chunk must be written (emission order) before phase C.
        for lc in range(8):
            v_unit(lc)
            if lc == 3:
                dma_xv(2)
            if lc == 7:
                dma_xv(3)
        for lc in range(8, NLK):
            v_unit(lc)

        # ---------------- filler queue ----------------
        def f_dma_xk2(lb):
            xk2[lb] = dma_xk(lb)

        # Filler queue in deadline order, with per-unit cost estimates so a
        # pop slot can take several small units but stops before
        # oversubscribing the PE between score matmuls.  qt[m][lb] is needed
        # at supertile 4m+lb; kt[m] fully at supertile 4m; k-m1 norms are
        # per-m so kt[1] is ready by st4 without waiting on m2/m3.
        # (cost_us, deadline_st, fn): deadline_st = supertile index before
        # which the filler MUST have been emitted (enforced by a force-pop at
        # each supertile start); pops otherwise drain the queue greedily
        # within a per-slot PE budget.
        F = []
        F += [(1.95, 1, lambda: proj_unit("k", 0, 1)),
              (0.8, 1, lambda: norm_unit("k", [0], 1)),
              (1.95, 1, lambda: proj_unit("q", 0, 1)),
              (1.95, 1, lambda: proj_unit("k", 0, 2)),
              (0.8, 1, lambda: norm_unit("k", [0], 2)),
              (0.8, 1, lambda: norm_unit("q", [0], 1)),
              (1.95, 1, lambda: proj_unit("k", 0, 3)),
              (0.8, 1, lambda: norm_unit("k", [0], 3)),
              (1.95, 2, lambda: proj_unit("q", 0, 2)),
              (0.8, 2, lambda: norm_unit("q", [0], 2)),
              (0.1, 3, lambda: f_dma_xk2(0)), (0.1, 3, lambda: f_dma_xk2(1)),
              (1.95, 4, lambda: proj_unit("q", 1, 0)),
              (1.95, 4, lambda: proj_unit("q", 2, 0)),
              (1.95, 4, lambda: proj_unit("q", 3, 0)),
              (0.1, 3, lambda: xq_t.__setitem__(3, dma_xq(3, nc.sync))),
              (1.95, 3, lambda: proj_unit("q", 0, 3)),
              (0.1, 4, lambda: f_dma_xk2(2)), (0.1, 4, lambda: f_dma_xk2(3)),
              (0.8, 3, lambda: norm_unit("q", [0], 3)),
              (0.8, 4, lambda: norm_unit("q", [1, 2, 3], 0))]
        F += [(1.95, 4, lambda lb=lb: proj_unit("k", 1, lb))
              for lb in range(NLB)]
        F += [(0.8, 4, lambda lb=lb: norm_unit("k", [1], lb))
              for lb in range(NLB)]
        F += [(1.95, 5, lambda: proj_unit("q", 1, 1)),
              (1.95, 5, lambda: proj_unit("q", 2, 1)),
              (1.95, 5, lambda: proj_unit("q", 3, 1)),
              (0.8, 5, lambda: norm_unit("q", [1, 2, 3], 1))]
        F += [(1.95, 8, lambda lb=lb: proj_unit("k", 2, lb))
              for lb in range(2)]
        F += [(1.95, 6, lambda: proj_unit("q", 1, 2)),
              (1.95, 6, lambda: proj_unit("q", 2, 2)),
              (1.95, 6, lambda: proj_unit("q", 3, 2)),
              (0.8, 6, lambda: norm_unit("q", [1, 2, 3], 2))]
        F += [(1.95, 8, lambda lb=lb: proj_unit("k", 2, lb))
              for lb in range(2, 4)]
        F += [(1.95, 7, lambda: proj_unit("q", 1, 3)),
              (1.95, 7, lambda: proj_unit("q", 2, 3)),
              (1.95, 7, lambda: proj_unit("q", 3, 3)),
              (0.8, 7, lambda: norm_unit("q", [1, 2, 3], 3))]
        F += [(1.95, 8, lambda lb=lb: proj_unit("k", 3, lb))
              for lb in range(NLB)]
        F += [(0.8, 8, lambda lb=lb: norm_unit("k", [2, 3], lb))
              for lb in range(NLB)]
        F += [(0.3, 13, dma_wo)]
        fillers = deque(F)

        def pop_fill(budget=1.3):
            while fillers:
                cost, _dl, fn = fillers[0]
                if cost > budget and budget < 1.3:
                    break
                fillers.popleft()
                fn()
                budget -= cost
                if budget <= 0:
                    break

        def force_pop(st_idx):
            while fillers and fillers[0][1] <= st_idx:
                fillers.popleft()[2]()

        # ---------------- phase C: 16 supertiles, m outer ----------------
        POP_LK = (1, 3, 5, 7, 9, 11, 13, 14)
        POP_LK_FINAL = tuple(range(1, 15))
        for m in range(NM):
            for lq in range(NLB):
                first = (m == 0 and lq == 0)
                pops = (POP_LK_FINAL if (m == NM - 1 or first) else POP_LK)
                force_pop(4 * m + lq)
                oa = None
                for lk in range(NLK):
                    if lk in pops:
                        pop_fill(budget=1.2 if m == NM - 1 else 1.3)
                    pss = psS.tile([P, 1024], F32, tag="pss")
                    # the two heads run in the two PE array halves
                    for s in range(2):
                        base = s * 64
                        nc.tensor.matmul(
                            pss[:, s * 512:(s + 1) * 512],
                            lhsT=kt[m][base:base + 64, lk * P:(lk + 1) * P],
                            rhs=qt[m][base:base + 64, lq * 512:(lq + 1) * 512],
                            start=True, stop=True)
                    et = etp.tile([P, 1024], BF16, tag="et")
                    nc.scalar.activation(out=et[:], in_=pss[:], func=EXP)
                    if lk == 0:
                        oa = [psO.tile([P, 260], F32, tag=f"oa{s}",
                                       name=f"oa{s}") for s in range(2)]
                    for s in range(2):
                        for c in range(4):
                            # start only on the bank's first matmul: a start
                            # marks the whole zero-region pending-zero, so
                            # later first-touches of the other column slices
                            # overwrite (not accumulate) as intended
                            nc.tensor.matmul(
                                oa[s][:, c * 65:(c + 1) * 65],
                                lhsT=et[:, s * 512 + c * P:s * 512 + (c + 1) * P],
                                rhs=vg[lk][:, 2 * m + s, :],
                                start=(lk == 0 and c == 0),
                                stop=(lk == NLK - 1),
                                skip_group_check=True)
                fillers.appendleft((1.0, 4 * m + lq + 1, make_drain(m, lq, oa)))
                if m == NM - 1:
                    for mo in range(D // P):
                        fillers.append((1.0, 99, make_op(mo, lq)))
        while fillers:
            pop_fill(budget=100.0)


def _patch_act_tables():
    """Steer the greedy activation-table assignment to the one table that
    holds BOTH ln and exp, so norm rsqrt (exp(-0.5*ln)) interleaves with the
    softmax exp stream without 1.3us table reloads.  Table order/ids are
    preserved; ln/exp are just removed from every other table."""
    if getattr(bacc, "_act_tables_patched", False):
        return
    orig = bacc.get_activation_tables

    def patched(arch):
        tabs = orig(arch)
        both = {n for n, fs in tabs.items()
                if any("Exp" == str(f).split(".")[-1] for f in fs)
                and any("Ln" == str(f).split(".")[-1] for f in fs)}
        if not both:
            return tabs
        keep = sorted(both)[0]
        out = {}
        for n, fs in tabs.items():
            if n == keep:
                out[n] = fs
            else:
                out[n] = {f for f in fs
                          if str(f).split(".")[-1] not in ("Exp", "Ln")}
        return out

    bacc.get_activation_tables = patched
    bacc._act_tables_patched = True


def build_nc(repeat=1):
    key = repeat
    if key in _CACHE:
        return _CACHE[key]
    if not os.environ.get("K_NO_TABPATCH"):
        _patch_act_tables()
    nc = bacc.Bacc("TRN2", target_bir_lowering=False, debug=False, num_devices=8)
    prm = {}
    for name in ("xq", "xk"):
        prm[name] = nc.declare_dram_parameter(name, [D, L], BF16, isOutput=False)
    prm["xv"] = nc.declare_dram_parameter("xv", [D, L], BF16, isOutput=False)
    for name in ("wq", "wk"):
        prm[name] = nc.declare_dram_parameter(name, [D, DO], BF16, isOutput=False)
    prm["wv"] = nc.declare_dram_parameter("wv", [D, DO], BF16, isOutput=False)
    prm["wo"] = nc.declare_dram_parameter("wo", [DO, D], BF16, isOutput=False)
    prm["bq"] = nc.declare_dram_parameter("bq", [DO], F32, isOutput=False)
    prm["bk"] = nc.declare_dram_parameter("bk", [DO], F32, isOutput=False)
    prm["indt"] = nc.declare_dram_parameter("indt", [P, 2], BF16, isOutput=False)
    prm["sel2q"] = nc.declare_dram_parameter("sel2q", [66, P], F32R, isOutput=False)
    prm["sel2k"] = nc.declare_dram_parameter("sel2k", [66, P], F32R, isOutput=False)
    prm["ident"] = nc.declare_dram_parameter("ident", [P, P], BF16, isOutput=False)
    prm["out_t"] = nc.declare_dram_parameter("out_t", [D, L], BF16, isOutput=True)
    _emit(nc, prm, repeat=repeat)
    nc.compile()
    _CACHE[key] = nc
    return nc


def _bf16(a):
    import ml_dtypes
    return np.ascontiguousarray(np.asarray(a, np.float32).astype(ml_dtypes.bfloat16))


def _e4(a):
    return np.ascontiguousarray(
        np.asarray(a, np.float32).astype(mybir.dt.np(mybir.dt.float8e4)))


def make_in_maps(q, k, v, Wq, bq, Wk, bk, Wv, bv, Wo, bo):
    B = q.shape[0]
    f32 = np.float32

    indt = np.zeros((P, 2), f32)
    indt[0:64, 0] = 1.0
    indt[64:128, 1] = 1.0
    indt = _bf16(indt)
    sel2q = np.zeros((66, P), f32)
    for base in (0, 32, 64):
        sel2q[base, 0:64] = 1.0
        sel2q[base + 1, 64:128] = 1.0
    sel2k = sel2q * (1.0 / TAU)
    ident = np.eye(P, dtype=f32)

    in_maps = []
    for b in range(B):
        for g in range(2):
            sl = slice(g * DO, (g + 1) * DO)
            in_maps.append({
                "xq": _bf16(q[b].T),
                "xk": _bf16(k[b].T),
                "xv": _bf16(v[b].T),
                "wq": _bf16(Wq[sl, :].T),
                "wk": _bf16(Wk[sl, :].T),
                "wv": _bf16(Wv[sl, :].T),
                "wo": _bf16(Wo[:, sl].T),
                "bq": np.ascontiguousarray(bq[sl].astype(f32)),
                "bk": np.ascontiguousarray(bk[sl].astype(f32)),
                "indt": indt, "sel2q": sel2q.astype(f32), "sel2k": sel2k.astype(f32),
                "ident": _bf16(ident),
            })
    return in_maps


def assemble(results, bv, Wo, bo):
    B = len(results) // 2
    bias = (bo + bv @ Wo.T).astype(np.float32)
    outs = []
    for b in range(B):
        part = (results[2 * b]["out_t"].astype(np.float32)
                + results[2 * b + 1]["out_t"].astype(np.float32))
        outs.append(part.T + bias)
    return np.stack(outs).astype(np.float32)


def kernel(q, k, v, Wq, bq, Wk, bk, Wv, bv, Wo, bo):
    q, k, v = (np.asarray(t, np.float32) for t in (q, k, v))
    Wq, bq, Wk, bk, Wv, bv, Wo, bo = (
        np.asarray(t, np.float32) for t in (Wq, bq, Wk, bk, Wv, bv, Wo, bo))
    nc = build_nc()
    in_maps = make_in_maps(q, k, v, Wq, bq, Wk, bk, Wv, bv, Wo, bo)
    last_err = None
    for attempt in range(3):
        try:
            res = run_bass_kernel_spmd(nc, in_maps, core_ids=list(range(8)))
            return assemble(res.results, bv, Wo, bo)
        except Exception as e:  # transient NRT device errors: retry
            last_err = e
            import time as _time
            _time.sleep(2.0)
    raise last_err
